# revision 2
# baseline (speedup 1.0000x reference)
"""Trainium2 Bass kernel for nn_AttentionLayer_10995116278518.

Computes softmax(einsum('sbe,e->bs', embedded, attn[:300])
              + einsum('sbf,f->bs', lstm_outputs, attn[300:]), axis=1)
(the reference's mask is computed-but-discarded, so it is unused here).

Sharding: data-parallel over batch. Each of the 8 cores handles 8 of the
64 batch rows; no cross-device communication.

The kernel is pure streaming (every input element is used exactly once),
so time == bytes / HBM-BW. The host casts both big inputs to fp16
(validated: end-to-end rel err 5.2e-3 vs the 2e-2 gate), halving HBM
traffic to ~36 MB/core (~101 us roofline at 358 GB/s).

Per-core device kernel: host pre-transposes the shards feature-major so
every dot product is a TensorE matmul with the contraction (feature)
dim on partitions. For feature-chunk c and batch row b:
    matmul(out=logits[8, 512], lhsT=e_b (x) attn_c [128, 8], rhs=x [128, 512])
where lhsT has attn_c in column b and zeros elsewhere, so each matmul
adds batch-b row-dots into row b of a single PSUM tile and adds zero to
the other rows. All 280 matmuls (32 lstm chunks + 3 zero-padded
embedded chunks, x 8 batch rows) accumulate into one PSUM bank that is
exactly the [8b, 512s] logits layout softmax wants: no transposes.
"""

import sys

import numpy as np

try:
    import concourse.bass as bass
except ImportError:  # stand-alone grading dir: the runtime lives here
    sys.path.insert(0, "/opt/trn_rl_repo")
    import concourse.bass as bass

import concourse.bacc as bacc
import concourse.tile as tile
from concourse import mybir
from concourse.bass_utils import run_bass_kernel_spmd

SEQ = 512
BATCH = 64
EMB = 300
EMB_PAD = 384  # zero-padded to 3 chunks of 128
LSTM = 4096
N_CORES = 8
BLOC = BATCH // N_CORES  # 8 batch rows per core
P = 128
NCL = LSTM // P  # 32 lstm feature chunks
NCE = EMB_PAD // P  # 3 embedded feature chunks
NC_ALL = NCL + NCE  # 35
NG = 4  # lstm chunk groups per batch row (8 chunks = 1 MB per DMA)
GJ = NCL // NG  # 8 chunks per group

F32 = mybir.dt.float32
F16 = mybir.dt.float16


def _build() -> bass.Bass:
    nc = bacc.Bacc()
    # lstm shard, feature-major fp16: [b, g, p, j, s], f = (8g+j)*128+p
    lstm = nc.declare_dram_parameter(
        "lstm_outputs", [BLOC, NG, P, GJ, SEQ], F16, isOutput=False
    )
    # embedded shard, feature-major fp16 (padded): [p, b, j, s], f = j*128+p
    emb = nc.declare_dram_parameter(
        "embedded", [P, BLOC, NCE, SEQ], F16, isOutput=False
    )
    # stationary matrices: attn_lhsT[p, c, b, :] = attn_chunk_c[p] * e_b
    attn_lhsT = nc.declare_dram_parameter(
        "attn_lhsT", [P, NC_ALL, BLOC, BLOC], F16, isOutput=False
    )
    out = nc.declare_dram_parameter("out", [BLOC, SEQ], F32, isOutput=True)

    with tile.TileContext(nc) as tc:
        with (
            tc.tile_pool(name="singles", bufs=1) as singles,
            tc.tile_pool(name="lstm_tiles", bufs=8) as lstm_pool,
            tc.tile_pool(name="psum", bufs=1, space="PSUM") as psum_pool,
        ):
            # stationary attn matrices lead the scalar ring: the first
            # matmul needs them
            sb_attn = singles.tile([P, NC_ALL, BLOC, BLOC], F16)
            nc.scalar.dma_start(out=sb_attn, in_=attn_lhsT[:, :, :, :])

            # logits [8b, 512s] accumulate in one PSUM bank
            logits = psum_pool.tile([BLOC, SEQ], F32, tag="ps")

            order = [(b, g) for b in range(BLOC) for g in range(NG)]
            NT = len(order)
            lstm_tiles = {}

            def issue_lstm_dma(t):
                b, g = order[t]
                lt = lstm_pool.tile([P, GJ, SEQ], F16, tag="lstm")
                eng = nc.sync if t % 2 == 0 else nc.scalar
                eng.dma_start(out=lt, in_=lstm[b, g])
                lstm_tiles[t] = lt

            PREFETCH = 8
            # prime the pipeline; the embedded halves ride along early so
            # the mid-stream embedded matmuls never wait
            issue_lstm_dma(0)
            issue_lstm_dma(1)
            sb_emb = singles.tile([P, BLOC, NCE, SEQ], F16)
            nc.sync.dma_start(
                out=sb_emb[:, 0 : BLOC // 2], in_=emb[:, 0 : BLOC // 2]
            )
            nc.scalar.dma_start(
                out=sb_emb[:, BLOC // 2 : BLOC], in_=emb[:, BLOC // 2 : BLOC]
            )
            for t in range(2, PREFETCH):
                issue_lstm_dma(t)

            def lstm_matmuls(t):
                b, g = order[t]
                lt = lstm_tiles.pop(t)
                for j in range(GJ):
                    c = GJ * g + j
                    nc.tensor.matmul(
                        out=logits,
                        lhsT=sb_attn[:, c, b, :],
                        rhs=lt[:, j, :],
                        start=(t == 0 and j == 0),
                        stop=(t == NT - 1 and j == GJ - 1),
                    )

            for t in range(NT // 2):
                if t + PREFETCH < NT:
                    issue_lstm_dma(t + PREFETCH)
                lstm_matmuls(t)

            # embedded contributions mid-stream: they gate nothing
            for b in range(BLOC):
                for j in range(NCE):
                    nc.tensor.matmul(
                        out=logits,
                        lhsT=sb_attn[:, NCL + j, b, :],
                        rhs=sb_emb[:, b, j, :],
                        start=False,
                        stop=False,
                    )

            for t in range(NT // 2, NT):
                if t + PREFETCH < NT:
                    issue_lstm_dma(t + PREFETCH)
                lstm_matmuls(t)

            # softmax along s (free axis)
            m = singles.tile([BLOC, 1], F32)
            nm = singles.tile([BLOC, 1], F32)
            ssum = singles.tile([BLOC, 1], F32)
            rec = singles.tile([BLOC, 1], F32)
            expt = singles.tile([BLOC, SEQ], F32)
            res = singles.tile([BLOC, SEQ], F32)
            nc.vector.reduce_max(out=m, in_=logits, axis=mybir.AxisListType.X)
            nc.vector.tensor_scalar_mul(nm, m, -1.0)
            nc.scalar.activation(
                out=expt,
                in_=logits,
                func=mybir.ActivationFunctionType.Exp,
                bias=nm,
                scale=1.0,
                accum_out=ssum,
            )
            nc.vector.reciprocal(rec, ssum)
            nc.vector.tensor_scalar_mul(res, expt, rec)
            nc.sync.dma_start(out=out[:, :], in_=res)

    nc.compile()
    return nc


_NC_CACHE = None


def _get_nc() -> bass.Bass:
    global _NC_CACHE
    if _NC_CACHE is None:
        _NC_CACHE = _build()
    return _NC_CACHE


def _make_in_maps(embedded, lstm_outputs, attn):
    embedded = np.asarray(embedded, dtype=np.float32)
    lstm_outputs = np.asarray(lstm_outputs, dtype=np.float32)
    attn = np.asarray(attn, dtype=np.float32)

    lstm16 = lstm_outputs.astype(np.float16)  # [512, 64, 4096]
    emb16 = embedded.astype(np.float16)  # [512, 64, 300]

    # stationary matrices, shared across cores
    vals = np.zeros((NC_ALL, P), dtype=np.float16)
    vals[:NCL] = attn[EMB:].astype(np.float16).reshape(NCL, P)
    ve = np.zeros(EMB_PAD, dtype=np.float16)
    ve[:EMB] = attn[:EMB].astype(np.float16)
    vals[NCL:] = ve.reshape(NCE, P)
    attn_lhsT = np.zeros((P, NC_ALL, BLOC, BLOC), dtype=np.float16)
    for b in range(BLOC):
        attn_lhsT[:, :, b, b] = vals.T

    in_maps = []
    for i in range(N_CORES):
        sl = slice(i * BLOC, (i + 1) * BLOC)
        # [512, 8, 4096] -> [b, f, s] -> [b, g, j, p, s] -> [b, g, p, j, s]
        shard_l = (
            lstm16[:, sl, :]
            .transpose(1, 2, 0)
            .reshape(BLOC, NG, GJ, P, SEQ)
            .transpose(0, 1, 3, 2, 4)
        )
        # [512, 8, 300] -> pad -> [f, b, s] -> [j, p, b, s] -> [p, b, j, s]
        shard_e = np.zeros((SEQ, BLOC, EMB_PAD), dtype=np.float16)
        shard_e[:, :, :EMB] = emb16[:, sl, :]
        shard_e = (
            shard_e.transpose(2, 1, 0)
            .reshape(NCE, P, BLOC, SEQ)
            .transpose(1, 2, 0, 3)
        )
        in_maps.append(
            {
                "lstm_outputs": np.ascontiguousarray(shard_l),
                "embedded": np.ascontiguousarray(shard_e),
                "attn_lhsT": attn_lhsT,
            }
        )
    return in_maps


def _run(embedded, lstm_outputs, attn, trace=False, **spmd_kwargs):
    nc = _get_nc()
    in_maps = _make_in_maps(embedded, lstm_outputs, attn)
    r = run_bass_kernel_spmd(
        nc, in_maps, core_ids=list(range(N_CORES)), trace=trace, **spmd_kwargs
    )
    out = np.concatenate([r.results[i]["out"] for i in range(N_CORES)], axis=0)
    return out, r


def kernel(embedded, lstm_outputs, attn, mask=None, **_ignored) -> np.ndarray:
    out, _ = _run(embedded, lstm_outputs, attn, trace=False)
    return out.astype(np.float32)
